# revision 25
# baseline (speedup 1.0000x reference)
"""Trainium2 Bass kernel for a 12-head attention block.

Problem (hardcoded): x [16, 1024, 768] f32, w_qkv [2304, 768], w_proj
[768, 768], b_proj [768].  out = proj(softmax(q k^T / sqrt(64)) v).

Sharding: pure data parallel over batch - 16 batches / 8 cores = 2
batches per core, no collectives.  All layout transposes happen on the
host: each core receives x^T [768, 2048] (bf16) and produces out^T
[768, 2048] (f32).

Per-core kernel, all matmuls in bf16 (1 cycle/row on the PE, fp32 PSUM
accumulation):
  A) qkv projection of batch 0: qT/kT [768(head-major), 1024], V in
     natural [j, d] layout per j-chunk, ones-augmented so the PV matmul
     also produces the softmax denominator l[i] in row 64.
  B) attention, one head at a time: S^T[j,i] = K^T.T @ Q^T -> exp on
     ACT (scale folded, bf16 out) -> O^T_aug[65,i] = V_aug.T @ P^T
     accumulated over j-chunks in PSUM.  The per-head PE deficit vs the
     ACT exp stream is filled by interleaving the NEXT batch's qkv
     matmuls (during attention of batch 0) / the PREVIOUS batch's
     projection matmuls (during attention of batch 1) into the PE
     queue, so the tensor engine never idles and stays at max p-state.
  C) o_ps is eagerly drained PSUM->SBUF (frees the PSUM bank for the
     next head, po pool has 1 buf), then r = 1/l on DVE, broadcast
     across 64 partitions on GPSIMD, multiply into ot (bf16).
  D) proj: out^T = w_proj^T.T @ O^T + b_proj (f32), DMA back to DRAM.

PSUM: S/qkv/proj share a 3-buf [128,1024] pool (6 banks) + o_ps
[65,1024] 1 buf (2 banks) = 8 banks.
"""

import numpy as np
import ml_dtypes
from contextlib import ExitStack

import concourse.bass as bass
import concourse.mybir as mybir
import concourse.tile as tile
from concourse import bacc
from concourse import bass_utils

F32 = mybir.dt.float32
BF = mybir.dt.bfloat16
F8 = mybir.dt.float8e4
DR = mybir.MatmulPerfMode.DoubleRow
EXP = mybir.ActivationFunctionType.Exp

# QKT in fp8e4m3 DoubleRow: 0.5 PE cycles/row instead of 1.  q/k are
# quantized to fp8 (measured end-to-end rel err ~1.2e-2 vs the 2e-2
# gate); P and V stay bf16 (fp8 there would blow the error budget).
USE_FP8_QKT = True

B, N, C = 16, 1024, 768
H, D = 12, 64
E = 3 * C
NCORES = 8
BL = B // NCORES          # batches per core
T = BL * N                # tokens per core
KC = C // 128             # feature chunks of 128
JC = N // 128             # token chunks of 128
SCALE = float(D) ** -0.5

_CACHE = {}


def _mm(nc, out, lhsT, rhs, **kw):
    nc.tensor.matmul(out, lhsT=lhsT, rhs=rhs, **kw)


def _build(ctx, tc):
    nc = tc.nc
    dram = ctx.enter_context(tc.tile_pool(name="dram", bufs=1, space="DRAM"))
    # x^T blocked: [kc, b, 128, N] bf16
    xT_d = dram.tile([KC, BL, 128, N], BF, kind="ExternalInput", name="xTb", uniquify=False)
    # w_qkv^T as per-kc slabs [kc, 128, 2304] bf16
    wqkv_d = dram.tile([KC, 128, E], BF, kind="ExternalInput", name="wqkvb", uniquify=False)
    # w_proj^T per-kc slabs [kc, 128, 768] bf16
    wproj_d = dram.tile([KC, 128, C], BF, kind="ExternalInput", name="wprojb", uniquify=False)
    bproj_d = dram.tile([C, 1], F32, kind="ExternalInput", name="bproj", uniquify=False)
    # out^T blocked: [oc, b, 128, N] f32
    outT_d = dram.tile([KC, BL, 128, N], F32, kind="ExternalOutput", name="outTb", uniquify=False)

    consts = ctx.enter_context(tc.tile_pool(name="consts", bufs=1))
    wqk_pool = ctx.enter_context(tc.tile_pool(name="wqk", bufs=KC))
    wv_pool = ctx.enter_context(tc.tile_pool(name="wv", bufs=KC))
    wp_pool = ctx.enter_context(tc.tile_pool(name="wp", bufs=KC))
    xt_pool = ctx.enter_context(tc.tile_pool(name="xt", bufs=2 * KC))
    qk_pool = ctx.enter_context(tc.tile_pool(name="qk", bufs=4))
    va_pool = ctx.enter_context(tc.tile_pool(name="va", bufs=2 * JC))
    pt_pool = ctx.enter_context(tc.tile_pool(name="pt", bufs=4))
    ot_pool = ctx.enter_context(tc.tile_pool(name="ot", bufs=2 * KC))
    os_pool = ctx.enter_context(tc.tile_pool(name="os", bufs=3))
    sm_pool = ctx.enter_context(tc.tile_pool(name="sm", bufs=2))
    rb_pool = ctx.enter_context(tc.tile_pool(name="rb", bufs=2))
    ob_pool = ctx.enter_context(tc.tile_pool(name="ob", bufs=2))
    ps_pool = ctx.enter_context(tc.tile_pool(name="ps", bufs=3, space="PSUM"))
    po_pool = ctx.enter_context(tc.tile_pool(name="po", bufs=2, space="PSUM"))

    vones_bf = consts.tile([128, H, 1], BF)
    nc.vector.memset(vones_bf, 1.0)
    bias_sb = consts.tile([128, KC], F32)
    nc.sync.dma_start(
        out=bias_sb, in_=bproj_d[:, 0].rearrange("(k p) -> p k", p=128)
    )

    xt = {}

    def load_x(b):
        for kc in range(KC):
            t = xt_pool.tile([128, N], BF, name=f"xt{b}_{kc}", tag="xt")
            nc.sync.dma_start(out=t, in_=xT_d[kc, b])
            xt[(b, kc)] = t

    # batch-0 x first: the first matmuls wait on these DMAs.
    load_x(0)
    # q/k weights resident: per-kc [128, 12, 128] tile, ONE wide DMA each
    wqk_t = []
    for kc in range(KC):
        t = wqk_pool.tile([128, 2 * KC, 128], BF, name=f"wqk{kc}", tag="wqk")
        nc.sync.dma_start(out=t.rearrange("p a b -> p (a b)"), in_=wqkv_d[kc, :, 0:2 * C])
        wqk_t.append(t)
    wqk = {(j, kc): wqk_t[kc][:, j, :] for j in range(2 * KC) for kc in range(KC)}
    # v weights per-kc [128, 6, 128]
    wv = []
    for kc in range(KC):
        wvt = wv_pool.tile([128, KC, 128], BF, name=f"wv{kc}", tag="wv")
        nc.sync.dma_start(out=wvt.rearrange("p a b -> p (a b)"), in_=wqkv_d[kc, :, 2 * C:3 * C])
        wv.append(wvt)
    wp = {}

    qt, kt, va, ot = {}, {}, {}, {}

    def emit_qk_chunk(b, which, mt):
        dest = qt[b] if which == 0 else kt[b]
        ps = ps_pool.tile([128, N], F32, name=f"psqk{b}_{which}_{mt}", tag="ps")
        for kc in range(KC):
            w = wqk[(which * KC + mt, kc)]
            for hf in range(2):
                _mm(nc, ps[:, hf * 512:(hf + 1) * 512],
                    w, xt[(b, kc)][:, hf * 512:(hf + 1) * 512],
                    start=(kc == 0), stop=(kc == KC - 1))
        if USE_FP8_QKT:
            # quad layout: chunk mt = (quad g = mt//2, d-half = mt%2); the
            # host permuted w_qkv so PSUM partitions are (head-in-quad, d')
            nc.vector.tensor_copy(out=dest[:, mt // 2, mt % 2, :], in_=ps)
        else:
            nc.vector.tensor_copy(out=dest[:, mt, :], in_=ps)

    def emit_v_chunk(b, jc):
        vps = ps_pool.tile([128, C], F32, name=f"vps{b}_{jc}", tag="ps")
        for kc in range(KC):
            xs = xt[(b, kc)][:, jc * 128:(jc + 1) * 128]
            wvf = wv[kc].rearrange("p a b -> p (a b)")
            _mm(nc, vps[:, 0:512], xs, wvf[:, 0:512],
                start=(kc == 0), stop=(kc == KC - 1))
            _mm(nc, vps[:, 512:C], xs, wvf[:, 512:C],
                start=(kc == 0), stop=(kc == KC - 1))
        vat = va_pool.tile([128, H, D + 1], BF, name=f"va{b}_{jc}", tag="va")
        nc.vector.tensor_copy(
            out=vat[:, :, 0:D], in_=vps.rearrange("p (h d) -> p h d", h=H)
        )
        nc.vector.tensor_copy(out=vat[:, :, D:D + 1], in_=vones_bf)
        va[(b, jc)] = vat

    def emit_proj_chunk(b, oc):
        pps = ps_pool.tile([128, N], F32, name=f"pps{b}_{oc}", tag="ps")
        for kc in range(KC):
            for hf in range(2):
                _mm(nc, pps[:, hf * 512:(hf + 1) * 512],
                    wp[(kc, oc)],
                    ot[(b, kc)][:, hf * 512:(hf + 1) * 512],
                    start=(kc == 0), stop=(kc == KC - 1))
        obt = ob_pool.tile([128, N], F32, name=f"ob{b}_{oc}", tag="ob")
        # per-hf bias+writeback: the final chunk's DMA tail is halved and
        # lands on two queues
        for hf in range(2):
            sl = slice(hf * 512, (hf + 1) * 512)
            nc.vector.tensor_scalar_add(out=obt[:, sl], in0=pps[:, sl],
                                        scalar1=bias_sb[:, oc:oc + 1])
            nc.sync.dma_start(out=outT_d[oc, b, :, sl], in_=obt[:, sl])

    def emit_head(b, h, fillers, pops=(3,)):
        mt, off = h // 2, (h % 2) * D
        if h % 2 == 0:
            ot[(b, mt)] = ot_pool.tile([128, N], BF, name=f"ot{b}_{mt}", tag="ot")
        # hf-split o_ps: one PSUM bank each, released independently so the
        # next head's PV isn't gated on a full-width drain
        o_ps = [po_pool.tile([D + 1, 512], F32, name=f"ops{b}_{h}_{hf}", tag="ops")
                for hf in range(2)]

        g, a = h // 4, h % 4

        def qkt(jc):
            s = ps_pool.tile([128, N], F32, name=f"sps{b}_{h}_{jc}", tag="ps")
            for hf in range(2):
                if USE_FP8_QKT:
                    # DoubleRow: contraction 64 = 2 k-tiles of 32 partitions
                    _mm(nc, s[:, hf * 512:(hf + 1) * 512],
                        kt[b][a * 32:(a + 1) * 32, g, :, jc * 128:(jc + 1) * 128],
                        qt[b][a * 32:(a + 1) * 32, g, :, hf * 512:(hf + 1) * 512],
                        perf_mode=DR, tile_position=(a * 32, 0))
                else:
                    _mm(nc, s[:, hf * 512:(hf + 1) * 512],
                        kt[b][off:off + D, mt, jc * 128:(jc + 1) * 128],
                        qt[b][off:off + D, mt, hf * 512:(hf + 1) * 512])
            return s

        s = qkt(0)
        for jc in range(JC):
            pt = pt_pool.tile([128, N], BF, name=f"pt{b}_{h}_{jc}", tag="pt")
            nc.scalar.activation(out=pt, in_=s, func=EXP, scale=SCALE)
            if jc + 1 < JC:
                s = qkt(jc + 1)
            if jc in pops and fillers:
                fillers.pop(0)()
            for hf in range(2):
                _mm(nc, o_ps[hf],
                    va[(b, jc)][:, h, :], pt[:, hf * 512:(hf + 1) * 512],
                    start=(jc == 0), stop=(jc == JC - 1))
        # l (= row 64) + eager drain per hf: frees each o_ps bank quickly
        l_sb = sm_pool.tile([1, N], F32, name=f"l{b}_{h}", tag="sm")
        osb = os_pool.tile([D, N], F32, name=f"osb{b}_{h}", tag="os")
        for hf in range(2):
            nc.vector.tensor_copy(out=l_sb[:, hf * 512:(hf + 1) * 512],
                                  in_=o_ps[hf][D:D + 1, :])
            nc.vector.tensor_copy(out=osb[:, hf * 512:(hf + 1) * 512],
                                  in_=o_ps[hf][0:D, :])
        nc.vector.reciprocal_approx_fast(out=l_sb, in_=l_sb)
        rb = rb_pool.tile([D, N], F32, name=f"rb{b}_{h}", tag="rb")
        nc.gpsimd.partition_broadcast(rb, l_sb, channels=D)
        nc.vector.tensor_mul(
            out=ot[(b, mt)][off:off + D, :], in0=osb, in1=rb
        )

    # ---- phase A prologue: V(b0) + q/k quad 0 only, then attention of b0
    # starts immediately; the remaining q/k chunks of b0, all of qkv(b1),
    # and proj(b0) are interleaved into the attention head loops as PE
    # filler so the tensor engine never waits on the ACT exp stream.
    qkshape = ([128, 3, 2, N], F8) if USE_FP8_QKT else ([128, KC, N], BF)
    qt[0] = qk_pool.tile(qkshape[0], qkshape[1], name="qt0", tag="qk")
    kt[0] = qk_pool.tile(qkshape[0], qkshape[1], name="kt0", tag="qk")
    for jc in range(JC):
        emit_v_chunk(0, jc)
    for mt in range(2):            # quad 0 (heads 0..3)
        emit_qk_chunk(0, 0, mt)
        emit_qk_chunk(0, 1, mt)

    # w_proj loads during attention(b0): DMA bandwidth is free here
    for kc in range(KC):
        t = wp_pool.tile([128, KC, 128], BF, name=f"wpk{kc}", tag="wp")
        nc.sync.dma_start(out=t.rearrange("p a b -> p (a b)"), in_=wproj_d[kc])
        for oc in range(KC):
            wp[(kc, oc)] = t[:, oc, :]

    load_x(1)
    qt[1] = qk_pool.tile(qkshape[0], qkshape[1], name="qt1", tag="qk")
    kt[1] = qk_pool.tile(qkshape[0], qkshape[1], name="kt1", tag="qk")
    fillers = []
    for mt in range(2, KC):        # quads 1,2 of b0 (heads 4..11)
        fillers.append(lambda m=mt: emit_qk_chunk(0, 0, m))
        fillers.append(lambda m=mt: emit_qk_chunk(0, 1, m))
    for which in range(2):
        for mt in range(KC):
            fillers.append(lambda w=which, m=mt: emit_qk_chunk(1, w, m))
    for jc in range(JC):
        fillers.append(lambda j=jc: emit_v_chunk(1, j))
    for h in range(H):
        # 30 filler pieces over 12 heads: 2 pops/head, 3 for the back half
        emit_head(0, h, fillers, pops=(2, 5) if h < 6 else (2, 4, 6))
    while fillers:
        fillers.pop(0)()

    # ---- attention b1, with proj(b0) interleaved as PE filler ----
    fillers = [lambda o=oc: emit_proj_chunk(0, o) for oc in range(KC)]
    for h in range(H):
        emit_head(1, h, fillers, pops=(3,))
    while fillers:
        fillers.pop(0)()

    # ---- proj b1 ----
    for oc in range(KC):
        emit_proj_chunk(1, oc)


def get_nc():
    if "nc" not in _CACHE:
        nc = bacc.Bacc(None, target_bir_lowering=False, debug=False)
        with tile.TileContext(nc) as tc:
            with ExitStack() as ctx:
                _build(ctx, tc)
        nc.compile()
        _CACHE["nc"] = nc
    return _CACHE["nc"]


def make_in_maps(x, w_qkv, w_proj, b_proj):
    x = np.asarray(x, dtype=np.float32)
    w_qkv = np.asarray(w_qkv, dtype=np.float32)
    w_proj = np.asarray(w_proj, dtype=np.float32)
    BF_NP = ml_dtypes.bfloat16
    if USE_FP8_QKT:
        # quad layout: chunk (g, half) holds rows (4g+a)*64 + half*32 + d'
        perm = np.array([(4 * g + a) * 64 + half * 32 + d
                         for g in range(3) for half in range(2)
                         for a in range(4) for d in range(32)])
        w_qkv = np.concatenate(
            [w_qkv[0:C][perm], w_qkv[C:2 * C][perm], w_qkv[2 * C:]], axis=0)
    # w_qkv^T [c, e] -> per-kc slabs [kc, 128, 2304] bf16
    wqkvb = np.ascontiguousarray(w_qkv.T.reshape(KC, 128, E)).astype(BF_NP)
    # w_proj^T [c, o] -> per-kc slabs [kc, 128, 768] bf16
    wprojb = np.ascontiguousarray(w_proj.T.reshape(KC, 128, C)).astype(BF_NP)
    bp = np.ascontiguousarray(b_proj.astype(np.float32).reshape(C, 1))
    in_maps = []
    for c in range(NCORES):
        # x^T [c, t] -> blocks [kc, b, 128, N] bf16
        xT = x[c * BL:(c + 1) * BL].reshape(T, C).T  # [768, 2048]
        xb = np.ascontiguousarray(
            xT.reshape(KC, 128, BL, N).transpose(0, 2, 1, 3)
        ).astype(BF_NP)
        in_maps.append({"xTb": xb, "wqkvb": wqkvb, "wprojb": wprojb, "bproj": bp})
    return in_maps


def assemble_out(results):
    outs = []
    for c in range(NCORES):
        ob = results[c]["outTb"]  # [oc, b, 128, N]
        oT = ob.transpose(0, 2, 1, 3).reshape(C, T)
        outs.append(np.ascontiguousarray(oT.T).reshape(BL, N, C))
    return np.concatenate(outs, axis=0).astype(np.float32)


def kernel(x, w_qkv, w_proj, b_proj):
    nc = get_nc()
    in_maps = make_in_maps(x, w_qkv, w_proj, b_proj)
    res = bass_utils.run_bass_kernel_spmd(nc, in_maps, core_ids=list(range(NCORES)))
    return assemble_out(res.results)


# revision 26
# speedup vs baseline: 1.0850x; 1.0850x over previous
"""Trainium2 Bass kernel for a 12-head attention block.

Problem (hardcoded): x [16, 1024, 768] f32, w_qkv [2304, 768], w_proj
[768, 768], b_proj [768].  out = proj(softmax(q k^T / sqrt(64)) v).

Sharding: pure data parallel over batch - 16 batches / 8 cores = 2
batches per core, no collectives.  All layout transposes happen on the
host: each core receives x^T [768, 2048] (bf16) and produces out^T
[768, 2048] (f32).

Per-core kernel, all matmuls in bf16 (1 cycle/row on the PE, fp32 PSUM
accumulation):
  A) qkv projection of batch 0: qT/kT [768(head-major), 1024], V in
     natural [j, d] layout per j-chunk, ones-augmented so the PV matmul
     also produces the softmax denominator l[i] in row 64.
  B) attention, one head at a time: S^T[j,i] = K^T.T @ Q^T -> exp on
     ACT (scale folded, bf16 out) -> O^T_aug[65,i] = V_aug.T @ P^T
     accumulated over j-chunks in PSUM.  The per-head PE deficit vs the
     ACT exp stream is filled by interleaving the NEXT batch's qkv
     matmuls (during attention of batch 0) / the PREVIOUS batch's
     projection matmuls (during attention of batch 1) into the PE
     queue, so the tensor engine never idles and stays at max p-state.
  C) o_ps is eagerly drained PSUM->SBUF (frees the PSUM bank for the
     next head, po pool has 1 buf), then r = 1/l on DVE, broadcast
     across 64 partitions on GPSIMD, multiply into ot (bf16).
  D) proj: out^T = w_proj^T.T @ O^T + b_proj (f32), DMA back to DRAM.

PSUM: S/qkv/proj share a 3-buf [128,1024] pool (6 banks) + o_ps
[65,1024] 1 buf (2 banks) = 8 banks.
"""

import numpy as np
import ml_dtypes
from contextlib import ExitStack

import concourse.bass as bass
import concourse.mybir as mybir
import concourse.tile as tile
from concourse import bacc
from concourse import bass_utils

F32 = mybir.dt.float32
BF = mybir.dt.bfloat16
F8 = mybir.dt.float8e4
DR = mybir.MatmulPerfMode.DoubleRow
EXP = mybir.ActivationFunctionType.Exp

# QKT in fp8e4m3 DoubleRow was tried and measured SLOWER on hardware:
# DoubleRow doubles contraction per pass, not the column rate, so a
# contraction-64 QKT gains nothing (291ns vs 258ns per matmul) while
# adding quantization error (1.17e-2 vs 4.2e-3).  Keep bf16.
USE_FP8_QKT = False

B, N, C = 16, 1024, 768
H, D = 12, 64
E = 3 * C
NCORES = 8
BL = B // NCORES          # batches per core
T = BL * N                # tokens per core
KC = C // 128             # feature chunks of 128
JC = N // 128             # token chunks of 128
SCALE = float(D) ** -0.5

_CACHE = {}


def _mm(nc, out, lhsT, rhs, **kw):
    nc.tensor.matmul(out, lhsT=lhsT, rhs=rhs, **kw)


def _build(ctx, tc):
    nc = tc.nc
    dram = ctx.enter_context(tc.tile_pool(name="dram", bufs=1, space="DRAM"))
    # x^T blocked: [kc, b, 128, N] bf16
    xT_d = dram.tile([KC, BL, 128, N], BF, kind="ExternalInput", name="xTb", uniquify=False)
    # w_qkv^T as per-kc slabs [kc, 128, 2304] bf16
    wqkv_d = dram.tile([KC, 128, E], BF, kind="ExternalInput", name="wqkvb", uniquify=False)
    # w_proj^T per-kc slabs [kc, 128, 768] bf16
    wproj_d = dram.tile([KC, 128, C], BF, kind="ExternalInput", name="wprojb", uniquify=False)
    bproj_d = dram.tile([C, 1], F32, kind="ExternalInput", name="bproj", uniquify=False)
    # out^T blocked: [oc, b, 128, N] f32
    outT_d = dram.tile([KC, BL, 128, N], F32, kind="ExternalOutput", name="outTb", uniquify=False)

    consts = ctx.enter_context(tc.tile_pool(name="consts", bufs=1))
    wqk_pool = ctx.enter_context(tc.tile_pool(name="wqk", bufs=KC))
    wv_pool = ctx.enter_context(tc.tile_pool(name="wv", bufs=KC))
    wp_pool = ctx.enter_context(tc.tile_pool(name="wp", bufs=KC))
    xt_pool = ctx.enter_context(tc.tile_pool(name="xt", bufs=2 * KC))
    qk_pool = ctx.enter_context(tc.tile_pool(name="qk", bufs=4))
    va_pool = ctx.enter_context(tc.tile_pool(name="va", bufs=2 * JC))
    pt_pool = ctx.enter_context(tc.tile_pool(name="pt", bufs=4))
    ot_pool = ctx.enter_context(tc.tile_pool(name="ot", bufs=2 * KC))
    os_pool = ctx.enter_context(tc.tile_pool(name="os", bufs=3))
    sm_pool = ctx.enter_context(tc.tile_pool(name="sm", bufs=2))
    rb_pool = ctx.enter_context(tc.tile_pool(name="rb", bufs=2))
    ob_pool = ctx.enter_context(tc.tile_pool(name="ob", bufs=2))
    ps_pool = ctx.enter_context(tc.tile_pool(name="ps", bufs=3, space="PSUM"))
    po_pool = ctx.enter_context(tc.tile_pool(name="po", bufs=2, space="PSUM"))

    vones_bf = consts.tile([128, H, 1], BF)
    nc.vector.memset(vones_bf, 1.0)
    bias_sb = consts.tile([128, KC], F32)
    nc.sync.dma_start(
        out=bias_sb, in_=bproj_d[:, 0].rearrange("(k p) -> p k", p=128)
    )

    xt = {}

    def load_x(b):
        for kc in range(KC):
            t = xt_pool.tile([128, N], BF, name=f"xt{b}_{kc}", tag="xt")
            nc.sync.dma_start(out=t, in_=xT_d[kc, b])
            xt[(b, kc)] = t

    # batch-0 x first: the first matmuls wait on these DMAs.
    load_x(0)
    # q/k weights resident: per-kc [128, 12, 128] tile, ONE wide DMA each
    wqk_t = []
    for kc in range(KC):
        t = wqk_pool.tile([128, 2 * KC, 128], BF, name=f"wqk{kc}", tag="wqk")
        nc.sync.dma_start(out=t.rearrange("p a b -> p (a b)"), in_=wqkv_d[kc, :, 0:2 * C])
        wqk_t.append(t)
    wqk = {(j, kc): wqk_t[kc][:, j, :] for j in range(2 * KC) for kc in range(KC)}
    # v weights per-kc [128, 6, 128]
    wv = []
    for kc in range(KC):
        wvt = wv_pool.tile([128, KC, 128], BF, name=f"wv{kc}", tag="wv")
        nc.sync.dma_start(out=wvt.rearrange("p a b -> p (a b)"), in_=wqkv_d[kc, :, 2 * C:3 * C])
        wv.append(wvt)
    wp = {}

    qt, kt, va, ot = {}, {}, {}, {}

    def emit_qk_chunk(b, which, mt):
        dest = qt[b] if which == 0 else kt[b]
        ps = ps_pool.tile([128, N], F32, name=f"psqk{b}_{which}_{mt}", tag="ps")
        for kc in range(KC):
            w = wqk[(which * KC + mt, kc)]
            for hf in range(2):
                _mm(nc, ps[:, hf * 512:(hf + 1) * 512],
                    w, xt[(b, kc)][:, hf * 512:(hf + 1) * 512],
                    start=(kc == 0), stop=(kc == KC - 1))
        if USE_FP8_QKT:
            # quad layout: chunk mt = (quad g = mt//2, d-half = mt%2); the
            # host permuted w_qkv so PSUM partitions are (head-in-quad, d')
            nc.vector.tensor_copy(out=dest[:, mt // 2, mt % 2, :], in_=ps)
        else:
            nc.vector.tensor_copy(out=dest[:, mt, :], in_=ps)

    def emit_v_chunk(b, jc):
        vps = ps_pool.tile([128, C], F32, name=f"vps{b}_{jc}", tag="ps")
        for kc in range(KC):
            xs = xt[(b, kc)][:, jc * 128:(jc + 1) * 128]
            wvf = wv[kc].rearrange("p a b -> p (a b)")
            _mm(nc, vps[:, 0:512], xs, wvf[:, 0:512],
                start=(kc == 0), stop=(kc == KC - 1))
            _mm(nc, vps[:, 512:C], xs, wvf[:, 512:C],
                start=(kc == 0), stop=(kc == KC - 1))
        vat = va_pool.tile([128, H, D + 1], BF, name=f"va{b}_{jc}", tag="va")
        nc.vector.tensor_copy(
            out=vat[:, :, 0:D], in_=vps.rearrange("p (h d) -> p h d", h=H)
        )
        nc.vector.tensor_copy(out=vat[:, :, D:D + 1], in_=vones_bf)
        va[(b, jc)] = vat

    def emit_proj_chunk(b, oc):
        pps = ps_pool.tile([128, N], F32, name=f"pps{b}_{oc}", tag="ps")
        for kc in range(KC):
            for hf in range(2):
                _mm(nc, pps[:, hf * 512:(hf + 1) * 512],
                    wp[(kc, oc)],
                    ot[(b, kc)][:, hf * 512:(hf + 1) * 512],
                    start=(kc == 0), stop=(kc == KC - 1))
        obt = ob_pool.tile([128, N], F32, name=f"ob{b}_{oc}", tag="ob")
        # per-hf bias+writeback: the final chunk's DMA tail is halved and
        # lands on two queues
        for hf in range(2):
            sl = slice(hf * 512, (hf + 1) * 512)
            nc.vector.tensor_scalar_add(out=obt[:, sl], in0=pps[:, sl],
                                        scalar1=bias_sb[:, oc:oc + 1])
            nc.sync.dma_start(out=outT_d[oc, b, :, sl], in_=obt[:, sl])

    def emit_head(b, h, fillers, pops=(3,)):
        mt, off = h // 2, (h % 2) * D
        if h % 2 == 0:
            ot[(b, mt)] = ot_pool.tile([128, N], BF, name=f"ot{b}_{mt}", tag="ot")
        # hf-split o_ps: one PSUM bank each, released independently so the
        # next head's PV isn't gated on a full-width drain
        o_ps = [po_pool.tile([D + 1, 512], F32, name=f"ops{b}_{h}_{hf}", tag="ops")
                for hf in range(2)]

        g, a = h // 4, h % 4

        def qkt(jc):
            s = ps_pool.tile([128, N], F32, name=f"sps{b}_{h}_{jc}", tag="ps")
            for hf in range(2):
                if USE_FP8_QKT:
                    # DoubleRow: contraction 64 = 2 k-tiles of 32 partitions
                    _mm(nc, s[:, hf * 512:(hf + 1) * 512],
                        kt[b][a * 32:(a + 1) * 32, g, :, jc * 128:(jc + 1) * 128],
                        qt[b][a * 32:(a + 1) * 32, g, :, hf * 512:(hf + 1) * 512],
                        perf_mode=DR, tile_position=(a * 32, 0))
                else:
                    _mm(nc, s[:, hf * 512:(hf + 1) * 512],
                        kt[b][off:off + D, mt, jc * 128:(jc + 1) * 128],
                        qt[b][off:off + D, mt, hf * 512:(hf + 1) * 512])
            return s

        s = qkt(0)
        for jc in range(JC):
            pt = pt_pool.tile([128, N], BF, name=f"pt{b}_{h}_{jc}", tag="pt")
            nc.scalar.activation(out=pt, in_=s, func=EXP, scale=SCALE)
            if jc + 1 < JC:
                s = qkt(jc + 1)
            if jc in pops and fillers:
                fillers.pop(0)()
            for hf in range(2):
                _mm(nc, o_ps[hf],
                    va[(b, jc)][:, h, :], pt[:, hf * 512:(hf + 1) * 512],
                    start=(jc == 0), stop=(jc == JC - 1))
        # l (= row 64) + eager drain per hf: frees each o_ps bank quickly
        l_sb = sm_pool.tile([1, N], F32, name=f"l{b}_{h}", tag="sm")
        osb = os_pool.tile([D, N], F32, name=f"osb{b}_{h}", tag="os")
        for hf in range(2):
            nc.vector.tensor_copy(out=l_sb[:, hf * 512:(hf + 1) * 512],
                                  in_=o_ps[hf][D:D + 1, :])
            nc.vector.tensor_copy(out=osb[:, hf * 512:(hf + 1) * 512],
                                  in_=o_ps[hf][0:D, :])
        nc.vector.reciprocal_approx_fast(out=l_sb, in_=l_sb)
        rb = rb_pool.tile([D, N], F32, name=f"rb{b}_{h}", tag="rb")
        nc.gpsimd.partition_broadcast(rb, l_sb, channels=D)
        nc.vector.tensor_mul(
            out=ot[(b, mt)][off:off + D, :], in0=osb, in1=rb
        )

    # ---- phase A prologue: V(b0) + q/k quad 0 only, then attention of b0
    # starts immediately; the remaining q/k chunks of b0, all of qkv(b1),
    # and proj(b0) are interleaved into the attention head loops as PE
    # filler so the tensor engine never waits on the ACT exp stream.
    qkshape = ([128, 3, 2, N], F8) if USE_FP8_QKT else ([128, KC, N], BF)
    qt[0] = qk_pool.tile(qkshape[0], qkshape[1], name="qt0", tag="qk")
    kt[0] = qk_pool.tile(qkshape[0], qkshape[1], name="kt0", tag="qk")
    for jc in range(JC):
        emit_v_chunk(0, jc)
    for mt in range(2):            # quad 0 (heads 0..3)
        emit_qk_chunk(0, 0, mt)
        emit_qk_chunk(0, 1, mt)

    # w_proj loads during attention(b0): DMA bandwidth is free here
    for kc in range(KC):
        t = wp_pool.tile([128, KC, 128], BF, name=f"wpk{kc}", tag="wp")
        nc.sync.dma_start(out=t.rearrange("p a b -> p (a b)"), in_=wproj_d[kc])
        for oc in range(KC):
            wp[(kc, oc)] = t[:, oc, :]

    load_x(1)
    qt[1] = qk_pool.tile(qkshape[0], qkshape[1], name="qt1", tag="qk")
    kt[1] = qk_pool.tile(qkshape[0], qkshape[1], name="kt1", tag="qk")
    fillers = []
    for mt in range(2, KC):        # quads 1,2 of b0 (heads 4..11)
        fillers.append(lambda m=mt: emit_qk_chunk(0, 0, m))
        fillers.append(lambda m=mt: emit_qk_chunk(0, 1, m))
    for which in range(2):
        for mt in range(KC):
            fillers.append(lambda w=which, m=mt: emit_qk_chunk(1, w, m))
    for jc in range(JC):
        fillers.append(lambda j=jc: emit_v_chunk(1, j))
    for h in range(H):
        # 30 filler pieces over 12 heads: 2 pops/head, 3 for the back half
        emit_head(0, h, fillers, pops=(2, 5) if h < 6 else (2, 4, 6))
    while fillers:
        fillers.pop(0)()

    # ---- attention b1, with proj(b0) interleaved as PE filler ----
    fillers = [lambda o=oc: emit_proj_chunk(0, o) for oc in range(KC)]
    for h in range(H):
        emit_head(1, h, fillers, pops=(3,))
    while fillers:
        fillers.pop(0)()

    # ---- proj b1 ----
    for oc in range(KC):
        emit_proj_chunk(1, oc)


def get_nc():
    if "nc" not in _CACHE:
        nc = bacc.Bacc(None, target_bir_lowering=False, debug=False)
        with tile.TileContext(nc) as tc:
            with ExitStack() as ctx:
                _build(ctx, tc)
        nc.compile()
        _CACHE["nc"] = nc
    return _CACHE["nc"]


def make_in_maps(x, w_qkv, w_proj, b_proj):
    x = np.asarray(x, dtype=np.float32)
    w_qkv = np.asarray(w_qkv, dtype=np.float32)
    w_proj = np.asarray(w_proj, dtype=np.float32)
    BF_NP = ml_dtypes.bfloat16
    if USE_FP8_QKT:
        # quad layout: chunk (g, half) holds rows (4g+a)*64 + half*32 + d'
        perm = np.array([(4 * g + a) * 64 + half * 32 + d
                         for g in range(3) for half in range(2)
                         for a in range(4) for d in range(32)])
        w_qkv = np.concatenate(
            [w_qkv[0:C][perm], w_qkv[C:2 * C][perm], w_qkv[2 * C:]], axis=0)
    # w_qkv^T [c, e] -> per-kc slabs [kc, 128, 2304] bf16
    wqkvb = np.ascontiguousarray(w_qkv.T.reshape(KC, 128, E)).astype(BF_NP)
    # w_proj^T [c, o] -> per-kc slabs [kc, 128, 768] bf16
    wprojb = np.ascontiguousarray(w_proj.T.reshape(KC, 128, C)).astype(BF_NP)
    bp = np.ascontiguousarray(b_proj.astype(np.float32).reshape(C, 1))
    in_maps = []
    for c in range(NCORES):
        # x^T [c, t] -> blocks [kc, b, 128, N] bf16
        xT = x[c * BL:(c + 1) * BL].reshape(T, C).T  # [768, 2048]
        xb = np.ascontiguousarray(
            xT.reshape(KC, 128, BL, N).transpose(0, 2, 1, 3)
        ).astype(BF_NP)
        in_maps.append({"xTb": xb, "wqkvb": wqkvb, "wprojb": wprojb, "bproj": bp})
    return in_maps


def assemble_out(results):
    outs = []
    for c in range(NCORES):
        ob = results[c]["outTb"]  # [oc, b, 128, N]
        oT = ob.transpose(0, 2, 1, 3).reshape(C, T)
        outs.append(np.ascontiguousarray(oT.T).reshape(BL, N, C))
    return np.concatenate(outs, axis=0).astype(np.float32)


def kernel(x, w_qkv, w_proj, b_proj):
    nc = get_nc()
    in_maps = make_in_maps(x, w_qkv, w_proj, b_proj)
    res = bass_utils.run_bass_kernel_spmd(nc, in_maps, core_ids=list(range(NCORES)))
    return assemble_out(res.results)


# revision 27
# speedup vs baseline: 1.1151x; 1.0277x over previous
"""Trainium2 Bass kernel for a 12-head attention block.

Problem (hardcoded): x [16, 1024, 768] f32, w_qkv [2304, 768], w_proj
[768, 768], b_proj [768].  out = proj(softmax(q k^T / sqrt(64)) v).

Sharding: pure data parallel over batch - 16 batches / 8 cores = 2
batches per core, no collectives.  All layout transposes happen on the
host: each core receives x^T [768, 2048] (bf16) and produces out^T
[768, 2048] (f32).

Per-core kernel, all matmuls in bf16 (1 cycle/row on the PE, fp32 PSUM
accumulation):
  A) qkv projection of batch 0: qT/kT [768(head-major), 1024], V in
     natural [j, d] layout per j-chunk, ones-augmented so the PV matmul
     also produces the softmax denominator l[i] in row 64.
  B) attention, one head at a time: S^T[j,i] = K^T.T @ Q^T -> exp on
     ACT (scale folded, bf16 out) -> O^T_aug[65,i] = V_aug.T @ P^T
     accumulated over j-chunks in PSUM.  The per-head PE deficit vs the
     ACT exp stream is filled by interleaving the NEXT batch's qkv
     matmuls (during attention of batch 0) / the PREVIOUS batch's
     projection matmuls (during attention of batch 1) into the PE
     queue, so the tensor engine never idles and stays at max p-state.
  C) o_ps is eagerly drained PSUM->SBUF (frees the PSUM bank for the
     next head, po pool has 1 buf), then r = 1/l on DVE, broadcast
     across 64 partitions on GPSIMD, multiply into ot (bf16).
  D) proj: out^T = w_proj^T.T @ O^T + b_proj (f32), DMA back to DRAM.

PSUM: S/qkv/proj share a 3-buf [128,1024] pool (6 banks) + o_ps
[65,1024] 1 buf (2 banks) = 8 banks.
"""

import numpy as np
import ml_dtypes
from contextlib import ExitStack

import concourse.bass as bass
import concourse.mybir as mybir
import concourse.tile as tile
from concourse import bacc
from concourse import bass_utils

F32 = mybir.dt.float32
BF = mybir.dt.bfloat16
F8 = mybir.dt.float8e4
DR = mybir.MatmulPerfMode.DoubleRow
EXP = mybir.ActivationFunctionType.Exp

# QKT in fp8e4m3 DoubleRow was tried and measured SLOWER on hardware:
# DoubleRow doubles contraction per pass, not the column rate, so a
# contraction-64 QKT gains nothing (291ns vs 258ns per matmul) while
# adding quantization error (1.17e-2 vs 4.2e-3).  Keep bf16.
USE_FP8_QKT = False

B, N, C = 16, 1024, 768
H, D = 12, 64
E = 3 * C
NCORES = 8
BL = B // NCORES          # batches per core
T = BL * N                # tokens per core
KC = C // 128             # feature chunks of 128
JC = N // 128             # token chunks of 128
SCALE = float(D) ** -0.5

_CACHE = {}


def _mm(nc, out, lhsT, rhs, **kw):
    nc.tensor.matmul(out, lhsT=lhsT, rhs=rhs, **kw)


def _build(ctx, tc):
    nc = tc.nc
    dram = ctx.enter_context(tc.tile_pool(name="dram", bufs=1, space="DRAM"))
    # x^T blocked: [kc, b, 128, N] bf16
    xT_d = dram.tile([KC, BL, 128, N], BF, kind="ExternalInput", name="xTb", uniquify=False)
    # w_qkv^T as per-kc slabs [kc, 128, 2304] bf16
    wqkv_d = dram.tile([KC, 128, E], BF, kind="ExternalInput", name="wqkvb", uniquify=False)
    # w_proj^T per-kc slabs [kc, 128, 768] bf16
    wproj_d = dram.tile([KC, 128, C], BF, kind="ExternalInput", name="wprojb", uniquify=False)
    bproj_d = dram.tile([C, 1], F32, kind="ExternalInput", name="bproj", uniquify=False)
    # out^T blocked: [oc, b, 128, N] f32
    outT_d = dram.tile([KC, BL, 128, N], F32, kind="ExternalOutput", name="outTb", uniquify=False)

    consts = ctx.enter_context(tc.tile_pool(name="consts", bufs=1))
    wqk_pool = ctx.enter_context(tc.tile_pool(name="wqk", bufs=KC))
    wv_pool = ctx.enter_context(tc.tile_pool(name="wv", bufs=KC))
    wp_pool = ctx.enter_context(tc.tile_pool(name="wp", bufs=KC))
    xt_pool = ctx.enter_context(tc.tile_pool(name="xt", bufs=2 * KC))
    qk_pool = ctx.enter_context(tc.tile_pool(name="qk", bufs=4))
    va_pool = ctx.enter_context(tc.tile_pool(name="va", bufs=2 * JC))
    pt_pool = ctx.enter_context(tc.tile_pool(name="pt", bufs=4))
    ot_pool = ctx.enter_context(tc.tile_pool(name="ot", bufs=2 * KC))
    os_pool = ctx.enter_context(tc.tile_pool(name="os", bufs=3))
    sm_pool = ctx.enter_context(tc.tile_pool(name="sm", bufs=2))
    rb_pool = ctx.enter_context(tc.tile_pool(name="rb", bufs=2))
    ob_pool = ctx.enter_context(tc.tile_pool(name="ob", bufs=2))
    ps_pool = ctx.enter_context(tc.tile_pool(name="ps", bufs=3, space="PSUM"))
    po_pool = ctx.enter_context(tc.tile_pool(name="po", bufs=2, space="PSUM"))

    vones_bf = consts.tile([128, H, 1], BF)
    nc.vector.memset(vones_bf, 1.0)
    bias_sb = consts.tile([128, KC], F32)
    nc.sync.dma_start(
        out=bias_sb, in_=bproj_d[:, 0].rearrange("(k p) -> p k", p=128)
    )

    xt = {}

    def load_x(b):
        for kc in range(KC):
            t = xt_pool.tile([128, N], BF, name=f"xt{b}_{kc}", tag="xt")
            nc.sync.dma_start(out=t, in_=xT_d[kc, b])
            xt[(b, kc)] = t

    # batch-0 x first: the first matmuls wait on these DMAs.
    load_x(0)
    # q/k weights resident: per-kc [128, 12, 128] tile, ONE wide DMA each
    wqk_t = []
    for kc in range(KC):
        t = wqk_pool.tile([128, 2 * KC, 128], BF, name=f"wqk{kc}", tag="wqk")
        nc.sync.dma_start(out=t.rearrange("p a b -> p (a b)"), in_=wqkv_d[kc, :, 0:2 * C])
        wqk_t.append(t)
    wqk = {(j, kc): wqk_t[kc][:, j, :] for j in range(2 * KC) for kc in range(KC)}
    # v weights per-kc [128, 6, 128]
    wv = []
    for kc in range(KC):
        wvt = wv_pool.tile([128, KC, 128], BF, name=f"wv{kc}", tag="wv")
        nc.sync.dma_start(out=wvt.rearrange("p a b -> p (a b)"), in_=wqkv_d[kc, :, 2 * C:3 * C])
        wv.append(wvt)
    wp = {}

    qt, kt, va, ot = {}, {}, {}, {}

    def emit_qk_chunk(b, which, mt):
        dest = qt[b] if which == 0 else kt[b]
        ps = ps_pool.tile([128, N], F32, name=f"psqk{b}_{which}_{mt}", tag="ps")
        for kc in range(KC):
            w = wqk[(which * KC + mt, kc)]
            for hf in range(2):
                _mm(nc, ps[:, hf * 512:(hf + 1) * 512],
                    w, xt[(b, kc)][:, hf * 512:(hf + 1) * 512],
                    start=(kc == 0), stop=(kc == KC - 1))
        if USE_FP8_QKT:
            # quad layout: chunk mt = (quad g = mt//2, d-half = mt%2); the
            # host permuted w_qkv so PSUM partitions are (head-in-quad, d')
            nc.vector.tensor_copy(out=dest[:, mt // 2, mt % 2, :], in_=ps)
        else:
            nc.vector.tensor_copy(out=dest[:, mt, :], in_=ps)

    def emit_v_chunk(b, jc):
        vps = ps_pool.tile([128, C], F32, name=f"vps{b}_{jc}", tag="ps")
        for kc in range(KC):
            xs = xt[(b, kc)][:, jc * 128:(jc + 1) * 128]
            wvf = wv[kc].rearrange("p a b -> p (a b)")
            _mm(nc, vps[:, 0:512], xs, wvf[:, 0:512],
                start=(kc == 0), stop=(kc == KC - 1))
            _mm(nc, vps[:, 512:C], xs, wvf[:, 512:C],
                start=(kc == 0), stop=(kc == KC - 1))
        vat = va_pool.tile([128, H, D + 1], BF, name=f"va{b}_{jc}", tag="va")
        nc.vector.tensor_copy(
            out=vat[:, :, 0:D], in_=vps.rearrange("p (h d) -> p h d", h=H)
        )
        nc.vector.tensor_copy(out=vat[:, :, D:D + 1], in_=vones_bf)
        va[(b, jc)] = vat

    def emit_proj_chunk(b, oc):
        pps = ps_pool.tile([128, N], F32, name=f"pps{b}_{oc}", tag="ps")
        for kc in range(KC):
            for hf in range(2):
                _mm(nc, pps[:, hf * 512:(hf + 1) * 512],
                    wp[(kc, oc)],
                    ot[(b, kc)][:, hf * 512:(hf + 1) * 512],
                    start=(kc == 0), stop=(kc == KC - 1))
        obt = ob_pool.tile([128, N], F32, name=f"ob{b}_{oc}", tag="ob")
        # per-hf bias+writeback: the final chunk's DMA tail is halved and
        # lands on two queues
        for hf in range(2):
            sl = slice(hf * 512, (hf + 1) * 512)
            nc.vector.tensor_scalar_add(out=obt[:, sl], in0=pps[:, sl],
                                        scalar1=bias_sb[:, oc:oc + 1])
            nc.sync.dma_start(out=outT_d[oc, b, :, sl], in_=obt[:, sl])

    def emit_head(b, h, fillers, pops=(3,)):
        mt, off = h // 2, (h % 2) * D
        if h % 2 == 0:
            ot[(b, mt)] = ot_pool.tile([128, N], BF, name=f"ot{b}_{mt}", tag="ot")
        # hf-split o_ps: one PSUM bank each, released independently so the
        # next head's PV isn't gated on a full-width drain
        o_ps = [po_pool.tile([D + 1, 512], F32, name=f"ops{b}_{h}_{hf}", tag="ops")
                for hf in range(2)]

        g, a = h // 4, h % 4

        def qkt(jc):
            s = ps_pool.tile([128, N], F32, name=f"sps{b}_{h}_{jc}", tag="ps")
            for hf in range(2):
                if USE_FP8_QKT:
                    # DoubleRow: contraction 64 = 2 k-tiles of 32 partitions
                    _mm(nc, s[:, hf * 512:(hf + 1) * 512],
                        kt[b][a * 32:(a + 1) * 32, g, :, jc * 128:(jc + 1) * 128],
                        qt[b][a * 32:(a + 1) * 32, g, :, hf * 512:(hf + 1) * 512],
                        perf_mode=DR, tile_position=(a * 32, 0))
                else:
                    _mm(nc, s[:, hf * 512:(hf + 1) * 512],
                        kt[b][off:off + D, mt, jc * 128:(jc + 1) * 128],
                        qt[b][off:off + D, mt, hf * 512:(hf + 1) * 512])
            return s

        s = qkt(0)
        for jc in range(JC):
            pt = pt_pool.tile([128, N], BF, name=f"pt{b}_{h}_{jc}", tag="pt")
            nc.scalar.activation(out=pt, in_=s, func=EXP, scale=SCALE)
            if jc + 1 < JC:
                s = qkt(jc + 1)
            if jc in pops and fillers:
                fillers.pop(0)()
            for hf in range(2):
                _mm(nc, o_ps[hf],
                    va[(b, jc)][:, h, :], pt[:, hf * 512:(hf + 1) * 512],
                    start=(jc == 0), stop=(jc == JC - 1))
        # l (= row 64) + eager drain per hf: frees each o_ps bank quickly
        l_sb = sm_pool.tile([1, N], F32, name=f"l{b}_{h}", tag="sm")
        osb = os_pool.tile([D, N], F32, name=f"osb{b}_{h}", tag="os")
        for hf in range(2):
            nc.vector.tensor_copy(out=l_sb[:, hf * 512:(hf + 1) * 512],
                                  in_=o_ps[hf][D:D + 1, :])
            nc.vector.tensor_copy(out=osb[:, hf * 512:(hf + 1) * 512],
                                  in_=o_ps[hf][0:D, :])
        nc.vector.reciprocal_approx_fast(out=l_sb, in_=l_sb)
        rb = rb_pool.tile([D, N], F32, name=f"rb{b}_{h}", tag="rb")
        nc.gpsimd.partition_broadcast(rb, l_sb, channels=D)
        nc.vector.tensor_mul(
            out=ot[(b, mt)][off:off + D, :], in0=osb, in1=rb
        )

    # ---- phase A: qkv of batch 0 ----
    qkshape = ([128, 3, 2, N], F8) if USE_FP8_QKT else ([128, KC, N], BF)
    qt[0] = qk_pool.tile(qkshape[0], qkshape[1], name="qt0", tag="qk")
    kt[0] = qk_pool.tile(qkshape[0], qkshape[1], name="kt0", tag="qk")
    for which in range(2):
        for mt in range(KC):
            emit_qk_chunk(0, which, mt)
    for jc in range(JC):
        emit_v_chunk(0, jc)

    # w_proj loads during attention(b0): DMA bandwidth is free here
    for kc in range(KC):
        t = wp_pool.tile([128, KC, 128], BF, name=f"wpk{kc}", tag="wp")
        nc.sync.dma_start(out=t.rearrange("p a b -> p (a b)"), in_=wproj_d[kc])
        for oc in range(KC):
            wp[(kc, oc)] = t[:, oc, :]

    # ---- attention b0, with qkv(b1) interleaved as PE filler ----
    load_x(1)
    qt[1] = qk_pool.tile(qkshape[0], qkshape[1], name="qt1", tag="qk")
    kt[1] = qk_pool.tile(qkshape[0], qkshape[1], name="kt1", tag="qk")
    fillers = []
    for which in range(2):
        for mt in range(KC):
            fillers.append(lambda w=which, m=mt: emit_qk_chunk(1, w, m))
    for jc in range(JC):
        fillers.append(lambda j=jc: emit_v_chunk(1, j))
    for h in range(H):
        emit_head(0, h, fillers, pops=(3,) if h < 4 else (3, 6))
    while fillers:
        fillers.pop(0)()

    # ---- attention b1, with proj(b0) interleaved as PE filler ----
    fillers = [lambda o=oc: emit_proj_chunk(0, o) for oc in range(KC)]
    for h in range(H):
        emit_head(1, h, fillers, pops=(3,))
    while fillers:
        fillers.pop(0)()

    # ---- proj b1 ----
    for oc in range(KC):
        emit_proj_chunk(1, oc)


def get_nc():
    if "nc" not in _CACHE:
        nc = bacc.Bacc(None, target_bir_lowering=False, debug=False)
        with tile.TileContext(nc) as tc:
            with ExitStack() as ctx:
                _build(ctx, tc)
        nc.compile()
        _CACHE["nc"] = nc
    return _CACHE["nc"]


def make_in_maps(x, w_qkv, w_proj, b_proj):
    x = np.asarray(x, dtype=np.float32)
    w_qkv = np.asarray(w_qkv, dtype=np.float32)
    w_proj = np.asarray(w_proj, dtype=np.float32)
    BF_NP = ml_dtypes.bfloat16
    if USE_FP8_QKT:
        # quad layout: chunk (g, half) holds rows (4g+a)*64 + half*32 + d'
        perm = np.array([(4 * g + a) * 64 + half * 32 + d
                         for g in range(3) for half in range(2)
                         for a in range(4) for d in range(32)])
        w_qkv = np.concatenate(
            [w_qkv[0:C][perm], w_qkv[C:2 * C][perm], w_qkv[2 * C:]], axis=0)
    # w_qkv^T [c, e] -> per-kc slabs [kc, 128, 2304] bf16
    wqkvb = np.ascontiguousarray(w_qkv.T.reshape(KC, 128, E)).astype(BF_NP)
    # w_proj^T [c, o] -> per-kc slabs [kc, 128, 768] bf16
    wprojb = np.ascontiguousarray(w_proj.T.reshape(KC, 128, C)).astype(BF_NP)
    bp = np.ascontiguousarray(b_proj.astype(np.float32).reshape(C, 1))
    in_maps = []
    for c in range(NCORES):
        # x^T [c, t] -> blocks [kc, b, 128, N] bf16
        xT = x[c * BL:(c + 1) * BL].reshape(T, C).T  # [768, 2048]
        xb = np.ascontiguousarray(
            xT.reshape(KC, 128, BL, N).transpose(0, 2, 1, 3)
        ).astype(BF_NP)
        in_maps.append({"xTb": xb, "wqkvb": wqkvb, "wprojb": wprojb, "bproj": bp})
    return in_maps


def assemble_out(results):
    outs = []
    for c in range(NCORES):
        ob = results[c]["outTb"]  # [oc, b, 128, N]
        oT = ob.transpose(0, 2, 1, 3).reshape(C, T)
        outs.append(np.ascontiguousarray(oT.T).reshape(BL, N, C))
    return np.concatenate(outs, axis=0).astype(np.float32)


def kernel(x, w_qkv, w_proj, b_proj):
    nc = get_nc()
    in_maps = make_in_maps(x, w_qkv, w_proj, b_proj)
    res = bass_utils.run_bass_kernel_spmd(nc, in_maps, core_ids=list(range(NCORES)))
    return assemble_out(res.results)


# revision 29
# speedup vs baseline: 1.1167x; 1.0014x over previous
"""Trainium2 Bass kernel for a 12-head attention block.

Problem (hardcoded): x [16, 1024, 768] f32, w_qkv [2304, 768], w_proj
[768, 768], b_proj [768].  out = proj(softmax(q k^T / sqrt(64)) v).

Sharding: pure data parallel over batch - 16 batches / 8 cores = 2
batches per core, no collectives.  All layout transposes happen on the
host: each core receives x^T [768, 2048] (bf16) and produces out^T
[768, 2048] (f32).

Per-core kernel, all matmuls in bf16 (1 cycle/row on the PE, fp32 PSUM
accumulation):
  A) qkv projection of batch 0: qT/kT [768(head-major), 1024], V in
     natural [j, d] layout per j-chunk, ones-augmented so the PV matmul
     also produces the softmax denominator l[i] in row 64.
  B) attention, one head at a time: S^T[j,i] = K^T.T @ Q^T -> exp on
     ACT (scale folded, bf16 out) -> O^T_aug[65,i] = V_aug.T @ P^T
     accumulated over j-chunks in PSUM.  The per-head PE deficit vs the
     ACT exp stream is filled by interleaving the NEXT batch's qkv
     matmuls (during attention of batch 0) / the PREVIOUS batch's
     projection matmuls (during attention of batch 1) into the PE
     queue, so the tensor engine never idles and stays at max p-state.
  C) o_ps is eagerly drained PSUM->SBUF (frees the PSUM bank for the
     next head, po pool has 1 buf), then r = 1/l on DVE, broadcast
     across 64 partitions on GPSIMD, multiply into ot (bf16).
  D) proj: out^T = w_proj^T.T @ O^T + b_proj (f32), DMA back to DRAM.

PSUM: S/qkv/proj share a 3-buf [128,1024] pool (6 banks) + o_ps
[65,1024] 1 buf (2 banks) = 8 banks.
"""

import numpy as np
import ml_dtypes
from contextlib import ExitStack

import concourse.bass as bass
import concourse.mybir as mybir
import concourse.tile as tile
from concourse import bacc
from concourse import bass_utils

F32 = mybir.dt.float32
BF = mybir.dt.bfloat16
F8 = mybir.dt.float8e4
DR = mybir.MatmulPerfMode.DoubleRow
EXP = mybir.ActivationFunctionType.Exp

# QKT in fp8e4m3 DoubleRow was tried and measured SLOWER on hardware:
# DoubleRow doubles contraction per pass, not the column rate, so a
# contraction-64 QKT gains nothing (291ns vs 258ns per matmul) while
# adding quantization error (1.17e-2 vs 4.2e-3).  Keep bf16.
USE_FP8_QKT = False

B, N, C = 16, 1024, 768
H, D = 12, 64
E = 3 * C
NCORES = 8
BL = B // NCORES          # batches per core
T = BL * N                # tokens per core
KC = C // 128             # feature chunks of 128
JC = N // 128             # token chunks of 128
SCALE = float(D) ** -0.5

_CACHE = {}


def _mm(nc, out, lhsT, rhs, **kw):
    nc.tensor.matmul(out, lhsT=lhsT, rhs=rhs, **kw)


def _build(ctx, tc):
    nc = tc.nc
    dram = ctx.enter_context(tc.tile_pool(name="dram", bufs=1, space="DRAM"))
    # x^T blocked: [kc, b, 128, N] bf16
    xT_d = dram.tile([KC, BL, 128, N], BF, kind="ExternalInput", name="xTb", uniquify=False)
    # w_qkv^T as per-kc slabs [kc, 128, 2304] bf16
    wqkv_d = dram.tile([KC, 128, E], BF, kind="ExternalInput", name="wqkvb", uniquify=False)
    # w_proj^T per-kc slabs [kc, 128, 768] bf16
    wproj_d = dram.tile([KC, 128, C], BF, kind="ExternalInput", name="wprojb", uniquify=False)
    bproj_d = dram.tile([C, 1], F32, kind="ExternalInput", name="bproj", uniquify=False)
    # out^T blocked: [oc, b, 128, N] f32
    outT_d = dram.tile([KC, BL, 128, N], F32, kind="ExternalOutput", name="outTb", uniquify=False)

    consts = ctx.enter_context(tc.tile_pool(name="consts", bufs=1))
    wqk_pool = ctx.enter_context(tc.tile_pool(name="wqk", bufs=KC))
    wv_pool = ctx.enter_context(tc.tile_pool(name="wv", bufs=KC))
    wp_pool = ctx.enter_context(tc.tile_pool(name="wp", bufs=KC))
    xt_pool = ctx.enter_context(tc.tile_pool(name="xt", bufs=2 * KC))
    qk_pool = ctx.enter_context(tc.tile_pool(name="qk", bufs=4))
    va_pool = ctx.enter_context(tc.tile_pool(name="va", bufs=2 * JC))
    pt_pool = ctx.enter_context(tc.tile_pool(name="pt", bufs=4))
    ot_pool = ctx.enter_context(tc.tile_pool(name="ot", bufs=2 * KC))
    os_pool = ctx.enter_context(tc.tile_pool(name="os", bufs=3))
    sm_pool = ctx.enter_context(tc.tile_pool(name="sm", bufs=2))
    rb_pool = ctx.enter_context(tc.tile_pool(name="rb", bufs=2))
    ob_pool = ctx.enter_context(tc.tile_pool(name="ob", bufs=2))
    ps_pool = ctx.enter_context(tc.tile_pool(name="ps", bufs=3, space="PSUM"))
    po_pool = ctx.enter_context(tc.tile_pool(name="po", bufs=2, space="PSUM"))

    vones_bf = consts.tile([128, H, 1], BF)
    nc.vector.memset(vones_bf, 1.0)
    bias_sb = consts.tile([128, KC], F32)
    nc.sync.dma_start(
        out=bias_sb, in_=bproj_d[:, 0].rearrange("(k p) -> p k", p=128)
    )

    xt = {}

    def load_x(b):
        for kc in range(KC):
            t = xt_pool.tile([128, N], BF, name=f"xt{b}_{kc}", tag="xt")
            nc.sync.dma_start(out=t, in_=xT_d[kc, b])
            xt[(b, kc)] = t

    # batch-0 x first: the first matmuls wait on these DMAs.  wv before
    # wqk: phase A computes V first, so the first matmul only needs
    # x + wv (2.6MB) instead of x + wqk (3.75MB).
    load_x(0)
    wv = []
    for kc in range(KC):
        wvt = wv_pool.tile([128, KC, 128], BF, name=f"wv{kc}", tag="wv")
        nc.sync.dma_start(out=wvt.rearrange("p a b -> p (a b)"), in_=wqkv_d[kc, :, 2 * C:3 * C])
        wv.append(wvt)
    wqk_t = []
    for kc in range(KC):
        t = wqk_pool.tile([128, 2 * KC, 128], BF, name=f"wqk{kc}", tag="wqk")
        nc.sync.dma_start(out=t.rearrange("p a b -> p (a b)"), in_=wqkv_d[kc, :, 0:2 * C])
        wqk_t.append(t)
    wqk = {(j, kc): wqk_t[kc][:, j, :] for j in range(2 * KC) for kc in range(KC)}
    wp = {}

    qt, kt, va, ot = {}, {}, {}, {}

    def emit_qk_chunk(b, which, mt):
        dest = qt[b] if which == 0 else kt[b]
        ps = ps_pool.tile([128, N], F32, name=f"psqk{b}_{which}_{mt}", tag="ps")
        for kc in range(KC):
            w = wqk[(which * KC + mt, kc)]
            for hf in range(2):
                _mm(nc, ps[:, hf * 512:(hf + 1) * 512],
                    w, xt[(b, kc)][:, hf * 512:(hf + 1) * 512],
                    start=(kc == 0), stop=(kc == KC - 1))
        if USE_FP8_QKT:
            # quad layout: chunk mt = (quad g = mt//2, d-half = mt%2); the
            # host permuted w_qkv so PSUM partitions are (head-in-quad, d')
            nc.vector.tensor_copy(out=dest[:, mt // 2, mt % 2, :], in_=ps)
        else:
            nc.vector.tensor_copy(out=dest[:, mt, :], in_=ps)

    def emit_v_chunk(b, jc):
        vps = ps_pool.tile([128, C], F32, name=f"vps{b}_{jc}", tag="ps")
        for kc in range(KC):
            xs = xt[(b, kc)][:, jc * 128:(jc + 1) * 128]
            wvf = wv[kc].rearrange("p a b -> p (a b)")
            _mm(nc, vps[:, 0:512], xs, wvf[:, 0:512],
                start=(kc == 0), stop=(kc == KC - 1))
            _mm(nc, vps[:, 512:C], xs, wvf[:, 512:C],
                start=(kc == 0), stop=(kc == KC - 1))
        vat = va_pool.tile([128, H, D + 1], BF, name=f"va{b}_{jc}", tag="va")
        nc.vector.tensor_copy(
            out=vat[:, :, 0:D], in_=vps.rearrange("p (h d) -> p h d", h=H)
        )
        nc.vector.tensor_copy(out=vat[:, :, D:D + 1], in_=vones_bf)
        va[(b, jc)] = vat

    def emit_proj_chunk(b, oc):
        pps = ps_pool.tile([128, N], F32, name=f"pps{b}_{oc}", tag="ps")
        for kc in range(KC):
            for hf in range(2):
                _mm(nc, pps[:, hf * 512:(hf + 1) * 512],
                    wp[(kc, oc)],
                    ot[(b, kc)][:, hf * 512:(hf + 1) * 512],
                    start=(kc == 0), stop=(kc == KC - 1))
        obt = ob_pool.tile([128, N], F32, name=f"ob{b}_{oc}", tag="ob")
        # per-hf bias+writeback: the final chunk's DMA tail is halved and
        # lands on two queues
        for hf in range(2):
            sl = slice(hf * 512, (hf + 1) * 512)
            nc.vector.tensor_scalar_add(out=obt[:, sl], in0=pps[:, sl],
                                        scalar1=bias_sb[:, oc:oc + 1])
            nc.sync.dma_start(out=outT_d[oc, b, :, sl], in_=obt[:, sl])

    def emit_head(b, h, fillers, pops=(3,)):
        mt, off = h // 2, (h % 2) * D
        if h % 2 == 0:
            ot[(b, mt)] = ot_pool.tile([128, N], BF, name=f"ot{b}_{mt}", tag="ot")
        # hf-split o_ps: one PSUM bank each, released independently so the
        # next head's PV isn't gated on a full-width drain
        o_ps = [po_pool.tile([D + 1, 512], F32, name=f"ops{b}_{h}_{hf}", tag="ops")
                for hf in range(2)]

        g, a = h // 4, h % 4

        def qkt(jc):
            s = ps_pool.tile([128, N], F32, name=f"sps{b}_{h}_{jc}", tag="ps")
            for hf in range(2):
                if USE_FP8_QKT:
                    # DoubleRow: contraction 64 = 2 k-tiles of 32 partitions
                    _mm(nc, s[:, hf * 512:(hf + 1) * 512],
                        kt[b][a * 32:(a + 1) * 32, g, :, jc * 128:(jc + 1) * 128],
                        qt[b][a * 32:(a + 1) * 32, g, :, hf * 512:(hf + 1) * 512],
                        perf_mode=DR, tile_position=(a * 32, 0))
                else:
                    _mm(nc, s[:, hf * 512:(hf + 1) * 512],
                        kt[b][off:off + D, mt, jc * 128:(jc + 1) * 128],
                        qt[b][off:off + D, mt, hf * 512:(hf + 1) * 512])
            return s

        s = qkt(0)
        for jc in range(JC):
            pt = pt_pool.tile([128, N], BF, name=f"pt{b}_{h}_{jc}", tag="pt")
            nc.scalar.activation(out=pt, in_=s, func=EXP, scale=SCALE)
            if jc + 1 < JC:
                s = qkt(jc + 1)
            if jc in pops and fillers:
                fillers.pop(0)()
            for hf in range(2):
                _mm(nc, o_ps[hf],
                    va[(b, jc)][:, h, :], pt[:, hf * 512:(hf + 1) * 512],
                    start=(jc == 0), stop=(jc == JC - 1))
        # l (= row 64) + eager drain per hf: frees each o_ps bank quickly
        l_sb = sm_pool.tile([1, N], F32, name=f"l{b}_{h}", tag="sm")
        osb = os_pool.tile([D, N], F32, name=f"osb{b}_{h}", tag="os")
        for hf in range(2):
            nc.vector.tensor_copy(out=l_sb[:, hf * 512:(hf + 1) * 512],
                                  in_=o_ps[hf][D:D + 1, :])
            nc.vector.tensor_copy(out=osb[:, hf * 512:(hf + 1) * 512],
                                  in_=o_ps[hf][0:D, :])
        nc.vector.reciprocal_approx_fast(out=l_sb, in_=l_sb)
        rb = rb_pool.tile([D, N], F32, name=f"rb{b}_{h}", tag="rb")
        nc.gpsimd.partition_broadcast(rb, l_sb, channels=D)
        nc.vector.tensor_mul(
            out=ot[(b, mt)][off:off + D, :], in0=osb, in1=rb
        )

    # ---- phase A: qkv of batch 0 ----
    qkshape = ([128, 3, 2, N], F8) if USE_FP8_QKT else ([128, KC, N], BF)
    qt[0] = qk_pool.tile(qkshape[0], qkshape[1], name="qt0", tag="qk")
    kt[0] = qk_pool.tile(qkshape[0], qkshape[1], name="kt0", tag="qk")
    for jc in range(JC):
        emit_v_chunk(0, jc)
    for which in range(2):
        for mt in range(KC):
            emit_qk_chunk(0, which, mt)

    # w_proj loads during attention(b0): DMA bandwidth is free here
    for kc in range(KC):
        t = wp_pool.tile([128, KC, 128], BF, name=f"wpk{kc}", tag="wp")
        nc.sync.dma_start(out=t.rearrange("p a b -> p (a b)"), in_=wproj_d[kc])
        for oc in range(KC):
            wp[(kc, oc)] = t[:, oc, :]

    # ---- attention b0, with qkv(b1) interleaved as PE filler ----
    load_x(1)
    qt[1] = qk_pool.tile(qkshape[0], qkshape[1], name="qt1", tag="qk")
    kt[1] = qk_pool.tile(qkshape[0], qkshape[1], name="kt1", tag="qk")
    fillers = []
    for which in range(2):
        for mt in range(KC):
            fillers.append(lambda w=which, m=mt: emit_qk_chunk(1, w, m))
    for jc in range(JC):
        fillers.append(lambda j=jc: emit_v_chunk(1, j))
    for h in range(H):
        emit_head(0, h, fillers, pops=(3,) if h < 4 else (3, 6))
    while fillers:
        fillers.pop(0)()

    # ---- attention b1, with proj(b0) interleaved as PE filler ----
    fillers = [lambda o=oc: emit_proj_chunk(0, o) for oc in range(KC)]
    for h in range(H):
        emit_head(1, h, fillers, pops=(3,))
    while fillers:
        fillers.pop(0)()

    # ---- proj b1 ----
    for oc in range(KC):
        emit_proj_chunk(1, oc)


def get_nc():
    if "nc" not in _CACHE:
        nc = bacc.Bacc(None, target_bir_lowering=False, debug=False)
        with tile.TileContext(nc) as tc:
            with ExitStack() as ctx:
                _build(ctx, tc)
        nc.compile()
        _CACHE["nc"] = nc
    return _CACHE["nc"]


def make_in_maps(x, w_qkv, w_proj, b_proj):
    x = np.asarray(x, dtype=np.float32)
    w_qkv = np.asarray(w_qkv, dtype=np.float32)
    w_proj = np.asarray(w_proj, dtype=np.float32)
    BF_NP = ml_dtypes.bfloat16
    if USE_FP8_QKT:
        # quad layout: chunk (g, half) holds rows (4g+a)*64 + half*32 + d'
        perm = np.array([(4 * g + a) * 64 + half * 32 + d
                         for g in range(3) for half in range(2)
                         for a in range(4) for d in range(32)])
        w_qkv = np.concatenate(
            [w_qkv[0:C][perm], w_qkv[C:2 * C][perm], w_qkv[2 * C:]], axis=0)
    # w_qkv^T [c, e] -> per-kc slabs [kc, 128, 2304] bf16
    wqkvb = np.ascontiguousarray(w_qkv.T.reshape(KC, 128, E)).astype(BF_NP)
    # w_proj^T [c, o] -> per-kc slabs [kc, 128, 768] bf16
    wprojb = np.ascontiguousarray(w_proj.T.reshape(KC, 128, C)).astype(BF_NP)
    bp = np.ascontiguousarray(b_proj.astype(np.float32).reshape(C, 1))
    in_maps = []
    for c in range(NCORES):
        # x^T [c, t] -> blocks [kc, b, 128, N] bf16
        xT = x[c * BL:(c + 1) * BL].reshape(T, C).T  # [768, 2048]
        xb = np.ascontiguousarray(
            xT.reshape(KC, 128, BL, N).transpose(0, 2, 1, 3)
        ).astype(BF_NP)
        in_maps.append({"xTb": xb, "wqkvb": wqkvb, "wprojb": wprojb, "bproj": bp})
    return in_maps


def assemble_out(results):
    outs = []
    for c in range(NCORES):
        ob = results[c]["outTb"]  # [oc, b, 128, N]
        oT = ob.transpose(0, 2, 1, 3).reshape(C, T)
        outs.append(np.ascontiguousarray(oT.T).reshape(BL, N, C))
    return np.concatenate(outs, axis=0).astype(np.float32)


def kernel(x, w_qkv, w_proj, b_proj):
    nc = get_nc()
    in_maps = make_in_maps(x, w_qkv, w_proj, b_proj)
    res = bass_utils.run_bass_kernel_spmd(nc, in_maps, core_ids=list(range(NCORES)))
    return assemble_out(res.results)
